# revision 1
# baseline (speedup 1.0000x reference)
"""Cross-attention kernel for Trainium2 (8 NeuronCores, SPMD data-parallel).

Problem: O = softmax(Q @ K^T) @ V with B=4, Lq=Lk=4096, D=64, fp32 (no
1/sqrt(d) scaling).

Sharding: 8 cores = 4 batches x 2 Lq-halves. Each core handles a
[2048, 64] Q shard against the full [4096, 64] K/V of its batch.
Independent outputs -> no collectives.

Per-core algorithm (layouts chosen so nothing is transposed on-chip):
  - Host supplies QT [64, 2048] / KT [64, 4096] in fp16 (D on partitions),
    duplicated on-chip across both partition halves so two k-chunks'
    score matmuls run concurrently in the PE array via row tiling
    (contraction is only 64 rows deep).
  - ST[k, q] = matmul(lhsT=KT chunk [64,128], rhs=QT [64,512]) -> PSUM.
  - PT = exp(ST) on the scalar engine, written as bf16 (no max
    subtraction: |scores| < ~50, exp fits fp32/bf16 range; fp16 P would
    underflow).  The scalar engine at 1 elem/cycle/lane is the kernel's
    bottleneck, so exp instructions are kept at 1024 free elements.
  - OT[65, q] += matmul(lhsT=VA chunk [128, 65] bf16, rhs=PT [128, 512]):
    VA = concat([V, ones], 1); rows 0..63 accumulate unnormalized output,
    row 64 the softmax denominator. PT is consumed directly as lhsT-free
    rhs - no transpose anywhere.
  - Normalize: fast-reciprocal of row 64, gpsimd partition-broadcast,
    multiply, DMA out OT [64, 2048]; host transposes back.
"""

import sys

for _p in ("/opt/trn_rl_repo", "/opt/pypackages"):
    if _p not in sys.path:
        sys.path.insert(0, _p)

from contextlib import ExitStack

import ml_dtypes
import numpy as np

import concourse.bacc as bacc
import concourse.mybir as mybir
import concourse.tile as tile
from concourse.bass_utils import run_bass_kernel_spmd

# Problem constants (hardcoded per contract).
B, LQ, LK, D = 4, 4096, 4096, 64
N_CORES = 8
LQ_SHARD = LQ * B // N_CORES  # 2048
QB = 1024  # q-block (exp instruction free-size; 2 PSUM banks)
NQB = LQ_SHARD // QB  # 2
KC = 128  # k-chunk (contraction tile for the PV matmul)
NKC = LK // KC  # 32
SL = 512  # matmul moving-dim slice (one PSUM bank)
NSL = QB // SL  # 2

F32 = mybir.dt.float32
F16 = mybir.dt.float16
BF16 = mybir.dt.bfloat16

BF16NP = ml_dtypes.bfloat16

PACK_S = True  # row-tile two k-chunks' score matmuls concurrently
FAST_RECIP = True  # approx+NR reciprocal (~2 ULP) instead of exact (~6.5us)

KT_PIECE = 512  # kt DMA piece width (cols); 4 k-chunks per piece
VA_PIECE = 8  # va DMA piece size in k-chunks


def _build_program():
    nc = bacc.Bacc(
        "TRN2",
        target_bir_lowering=False,
        debug=False,
        num_devices=N_CORES,
    )
    qt_d = nc.declare_dram_parameter("QT", [D, LQ_SHARD], F16, isOutput=False)
    kt_d = nc.declare_dram_parameter("KT", [D, LK], F16, isOutput=False)
    va_d = nc.declare_dram_parameter("VA", [LK, D + 1], BF16, isOutput=False)
    ot_d = nc.declare_dram_parameter("OT", [D, LQ_SHARD], F32, isOutput=True)

    with tile.TileContext(nc) as tc, ExitStack() as ctx:
        singles = ctx.enter_context(tc.tile_pool(name="singles", bufs=1))
        st_pool = ctx.enter_context(tc.tile_pool(name="st", bufs=2, space="PSUM"))
        ot_pool = ctx.enter_context(tc.tile_pool(name="ot", bufs=2, space="PSUM"))
        pt_pool = ctx.enter_context(tc.tile_pool(name="pt", bufs=3))
        out_pool = ctx.enter_context(tc.tile_pool(name="out", bufs=2))
        norm_pool = ctx.enter_context(tc.tile_pool(name="norm", bufs=4))

        # Preload the exp activation table while input DMAs run.
        warm = singles.tile([1, 2], F32)
        nc.vector.memset(warm[:, :], 0.0)
        nc.scalar.activation(
            out=warm[:, :], in_=warm[:, :],
            func=mybir.ActivationFunctionType.Exp,
        )

        # QT/KT duplicated across both partition halves for PE row tiling.
        # Inputs are split into halves (separate tiles) so the first score
        # matmuls don't wait for the full 2 MB of loads; keeping the piece
        # count low preserves the Tile scheduler's pairing of the row-tiled
        # matmuls (many small tiles reorder the PE stream and let HAM
        # re-throttle the PE clock).
        va_r = va_d[:, :].rearrange("(c p) d -> p c d", p=KC)
        KH = LK // 2  # kt half width
        VH = NKC // 2  # va half size in chunks
        kt_sb = []
        qt_sb = []
        va_sb = []
        for h in range(2):
            tq = singles.tile([2 * D, QB], F16, name=f"qt{h}")
            sq = slice(h * QB, (h + 1) * QB)
            nc.sync.dma_start(out=tq[0:D, :], in_=qt_d[:, sq])
            nc.sync.dma_start(out=tq[D : 2 * D, :], in_=qt_d[:, sq])
            qt_sb.append(tq)
            t = singles.tile([2 * D, KH], F16, name=f"kt{h}")
            sl = slice(h * KH, (h + 1) * KH)
            nc.sync.dma_start(out=t[0:D, :], in_=kt_d[:, sl])
            nc.sync.dma_start(out=t[D : 2 * D, :], in_=kt_d[:, sl])
            kt_sb.append(t)
            tv = singles.tile([KC, VH, D + 1], BF16, name=f"va{h}")
            nc.sync.dma_start(
                out=tv[:, :, :], in_=va_r[:, h * VH : (h + 1) * VH, :]
            )
            va_sb.append(tv)

        def kt_ap(half, c):
            # [64, 128] fp16 weights for chunk c from partition half `half`
            t = kt_sb[c * KC // KH]
            off = (c * KC) % KH
            return t[half * D : (half + 1) * D, off : off + KC]

        def va_ap(c):
            return va_sb[c // VH][:, c % VH, :]

        for qb in range(NQB):
            ot_ps = ot_pool.tile([D + 1, QB], F32)
            for cp in range(NKC // 2):  # chunk pairs, row-tiled in the PE
                c0, c1 = 2 * cp, 2 * cp + 1
                st_a = st_pool.tile([KC, QB], F32, tag="st")
                st_b = st_pool.tile([KC, QB], F32, tag="st")
                for s in range(NSL):
                    q0 = qb * QB + s * SL
                    qt = qt_sb[qb]
                    if PACK_S:
                        nc.tensor.matmul(
                            out=st_a[:, s * SL : (s + 1) * SL],
                            lhsT=kt_ap(0, c0),
                            rhs=qt[0:D, s * SL : (s + 1) * SL],
                            start=True,
                            stop=True,
                            tile_position=(0, 0),
                        )
                        nc.tensor.matmul(
                            out=st_b[:, s * SL : (s + 1) * SL],
                            lhsT=kt_ap(1, c1),
                            rhs=qt[D : 2 * D, s * SL : (s + 1) * SL],
                            start=True,
                            stop=True,
                            tile_position=(D, 0),
                        )
                    else:
                        nc.tensor.matmul(
                            out=st_a[:, s * SL : (s + 1) * SL],
                            lhsT=kt_ap(0, c0),
                            rhs=qt[0:D, s * SL : (s + 1) * SL],
                            start=True,
                            stop=True,
                        )
                        nc.tensor.matmul(
                            out=st_b[:, s * SL : (s + 1) * SL],
                            lhsT=kt_ap(0, c1),
                            rhs=qt[0:D, s * SL : (s + 1) * SL],
                            start=True,
                            stop=True,
                        )
                for c, st_ps in ((c0, st_a), (c1, st_b)):
                    pt = pt_pool.tile([KC, QB], BF16)
                    nc.scalar.activation(
                        out=pt[:, :],
                        in_=st_ps[:, :],
                        func=mybir.ActivationFunctionType.Exp,
                    )
                    for s in range(NSL):
                        nc.tensor.matmul(
                            out=ot_ps[:, s * SL : (s + 1) * SL],
                            lhsT=va_ap(c),
                            rhs=pt[:, s * SL : (s + 1) * SL],
                            start=(c == 0),
                            stop=(c == NKC - 1),
                        )
            # Normalize: O[d, q] = OT[d, q] / OT[64, q]
            recip = norm_pool.tile([1, QB], F32)
            if FAST_RECIP:
                den = norm_pool.tile([1, QB], F32)
                nc.vector.tensor_copy(den[:, :], ot_ps[D : D + 1, :])
                scratch = norm_pool.tile([1, QB], F32)
                nc.vector.reciprocal_approx_accurate(
                    recip[:, :], den[:, :], scratch[:, :]
                )
            else:
                nc.vector.reciprocal(out=recip[:, :], in_=ot_ps[D : D + 1, :])
            bcast = norm_pool.tile([D, QB], F32)
            nc.gpsimd.partition_broadcast(bcast[:, :], recip[:, :])
            o_sb = out_pool.tile([D, QB], F32)
            nc.vector.tensor_mul(o_sb[:, :], ot_ps[0:D, :], bcast[:, :])
            nc.sync.dma_start(
                out=ot_d[:, qb * QB : (qb + 1) * QB], in_=o_sb[:, :]
            )

    nc.finalize()
    return nc


_PROGRAM_CACHE = {}


def _get_program():
    if "nc" not in _PROGRAM_CACHE:
        _PROGRAM_CACHE["nc"] = _build_program()
    return _PROGRAM_CACHE["nc"]


def _make_in_maps(Q, K, V):
    Q = np.asarray(Q, dtype=np.float32)
    K = np.asarray(K, dtype=np.float32)
    V = np.asarray(V, dtype=np.float32)
    in_maps = []
    ones = np.ones((LK, 1), dtype=np.float32)
    for core in range(N_CORES):
        b, half = core // 2, core % 2
        q_shard = Q[b, half * LQ_SHARD : (half + 1) * LQ_SHARD, :]  # [2048, 64]
        qt = np.ascontiguousarray(q_shard.T).astype(np.float16)  # [64, 2048]
        kt = np.ascontiguousarray(K[b].T).astype(np.float16)  # [64, 4096]
        va = np.concatenate([V[b], ones], axis=1).astype(BF16NP)  # [4096, 65]
        in_maps.append({"QT": qt, "KT": kt, "VA": np.ascontiguousarray(va)})
    return in_maps


def _run(Q, K, V, trace=False, **spmd_kwargs):
    nc = _get_program()
    in_maps = _make_in_maps(Q, K, V)
    res = run_bass_kernel_spmd(
        nc, in_maps, list(range(N_CORES)), trace=trace, **spmd_kwargs
    )
    out = np.empty((B, LQ, D), dtype=np.float32)
    for core in range(N_CORES):
        b, half = core // 2, core % 2
        ot = res.results[core]["OT"]  # [64, 2048]
        out[b, half * LQ_SHARD : (half + 1) * LQ_SHARD, :] = ot.T
    return out, res


def kernel(Q, K, V):
    out, _ = _run(Q, K, V, trace=False)
    return out



# revision 3
# speedup vs baseline: 1.1949x; 1.1949x over previous
"""Cross-attention kernel for Trainium2 (8 NeuronCores, SPMD data-parallel).

Problem: O = softmax(Q @ K^T) @ V with B=4, Lq=Lk=4096, D=64, fp32 (no
1/sqrt(d) scaling).

Sharding: 8 cores = 4 batches x 2 Lq-halves. Each core handles a
[2048, 64] Q shard against the full [4096, 64] K/V of its batch.
Independent outputs -> no collectives.

Per-core pipeline (one unit = one k-chunk of 128 keys x 1024 q):
  - ST[k, q] = matmul(lhsT=KT chunk [64,128] fp16, rhs=QT [64,512] fp16)
    -> PSUM [128, 1024] (2 banks, double-buffered).
  - P = exp(ST) -> bf16 SBUF [128, 1024]. Units alternate between the
    scalar engine (exact table exp) and the vector engine (Schraudolph
    bit-trick: int16(A*s + B) reinterpreted as bf16 ~= e^s), so neither
    engine's per-unit time exceeds the tensor engine's. The tensor
    engine must never stall: the cost model only grants the fast PE
    clock after 3us of gap-free execution.
  - O^T-free PV: out[q, d] += matmul(lhsT=PT [128,128], rhs=V chunk
    [128, 64]); denominator via rhs=ones [128, 1]. Output has q on
    partitions, so the PV matmuls stream only 64+1 columns per q-block
    instead of 512.
  - Warm-up matmuls on zeroed tiles run during the input-DMA head so the
    PE clock is already ramped when real work arrives.
  - Normalization (divide by denominator) happens on host after DMA-out,
    like the host-side transposes.
"""

import sys

for _p in ("/opt/trn_rl_repo", "/opt/pypackages"):
    if _p not in sys.path:
        sys.path.insert(0, _p)

from contextlib import ExitStack

import ml_dtypes
import numpy as np

import concourse.bacc as bacc
import concourse.mybir as mybir
import concourse.tile as tile
from concourse.bass_utils import run_bass_kernel_spmd

# Problem constants (hardcoded per contract).
B, LQ, LK, D = 4, 4096, 4096, 64
N_CORES = 8
LQ_SHARD = LQ * B // N_CORES  # 2048
KC = 128  # k-chunk (PV contraction tile)
NKC = LK // KC  # 32
QB = 1024  # q extent per unit (exp instruction free-size)
NQB = LQ_SHARD // QB  # 2
SL = 512  # score matmul moving-dim slice (one PSUM bank)
NSL = QB // SL  # 2
NJ = QB // 128  # q-sub-blocks per unit for the PV matmuls (8)

F32 = mybir.dt.float32
F16 = mybir.dt.float16
BF16 = mybir.dt.bfloat16
I16 = mybir.dt.int16

BF16NP = ml_dtypes.bfloat16

# Schraudolph constants for bf16: int16(A*s + B) bits viewed as bf16 ~ e^s.
SCH_A = float(128.0 / np.log(2.0))  # 184.664...
SCH_C = 8.0  # sawtooth centering shift
SCH_B = 128.0 * 127.0 - SCH_C + 0.5  # +0.5: float->int16 cast truncates

N_WARMUP_MM = 10  # PE clock ramp-up matmuls during the DMA head
N_SCALAR_UNITS = 33  # of 64 units; rest use the DVE Schraudolph exp


def _unit_on_scalar(u):
    # Spread 33 scalar units evenly among 64 (every other unit + one extra).
    return u % 2 == 0 or u == 1


def _build_program():
    nc = bacc.Bacc(
        "TRN2",
        target_bir_lowering=False,
        debug=False,
        num_devices=N_CORES,
    )
    qt_d = nc.declare_dram_parameter("QT", [D, LQ_SHARD], F16, isOutput=False)
    kt_d = nc.declare_dram_parameter("KT", [D, LK], F16, isOutput=False)
    vr_d = nc.declare_dram_parameter("VR", [KC, NKC, D], BF16, isOutput=False)
    o_d = nc.declare_dram_parameter("O", [KC, NQB * NJ * D], F32, isOutput=True)
    den_d = nc.declare_dram_parameter("DEN", [KC, NQB * NJ], F32, isOutput=True)

    with tile.TileContext(nc) as tc, ExitStack() as ctx:
        singles = ctx.enter_context(tc.tile_pool(name="singles", bufs=1))
        st_pool = ctx.enter_context(tc.tile_pool(name="st", bufs=2, space="PSUM"))
        ot_pool = ctx.enter_context(tc.tile_pool(name="ot", bufs=1, space="PSUM"))
        pt_pool = ctx.enter_context(tc.tile_pool(name="pt", bufs=4))
        ob_pool = ctx.enter_context(tc.tile_pool(name="ob", bufs=2))

        # Preload the exp activation table while input DMAs run.
        warm = singles.tile([1, 2], F32)
        nc.vector.memset(warm[:, :], 0.0)
        nc.scalar.activation(
            out=warm[:, :], in_=warm[:, :],
            func=mybir.ActivationFunctionType.Exp,
        )

        ones = singles.tile([KC, 1], BF16)
        nc.vector.memset(ones[:, :], 1.0)
        wt = singles.tile([D, 128 + SL], F16)
        nc.vector.memset(wt[:, :], 0.0)

        # Input DMAs, earliest-needed first.
        qt = singles.tile([D, LQ_SHARD], F16, name="qt")
        kt = singles.tile([D, LK], F16, name="kt")
        vr = singles.tile([KC, NKC, D], BF16, name="vr")
        KP = 4  # kt/vr DMA pieces
        nc.sync.dma_start(out=qt[:, 0:QB], in_=qt_d[:, 0:QB])
        for h in range(KP):
            kw = LK // KP
            nc.sync.dma_start(
                out=kt[:, h * kw : (h + 1) * kw], in_=kt_d[:, h * kw : (h + 1) * kw]
            )
            cw = NKC // KP
            nc.sync.dma_start(
                out=vr[:, h * cw : (h + 1) * cw, :],
                in_=vr_d[:, h * cw : (h + 1) * cw, :],
            )
        nc.sync.dma_start(out=qt[:, QB:], in_=qt_d[:, QB:])

        # PE clock ramp-up on zeroed tiles (overlaps the DMA head).
        warm_ps = ot_pool.tile([KC, SL], F32, name="warm_ps")
        for _ in range(N_WARMUP_MM):
            nc.tensor.matmul(
                out=warm_ps[:, :],
                lhsT=wt[:, 0:KC],
                rhs=wt[:, KC : KC + SL],
                start=True,
                stop=True,
            )

        ot = [ot_pool.tile([KC, NJ * D], F32, name=f"ot{qh}") for qh in range(NQB)]
        den_ps = ot_pool.tile([KC, NQB * NJ], F32, name="den_ps")

        for qh in range(NQB):
            for c in range(NKC):
                u = qh * NKC + c
                st = st_pool.tile([KC, QB], F32, tag="st")
                for s in range(NSL):
                    nc.tensor.matmul(
                        out=st[:, s * SL : (s + 1) * SL],
                        lhsT=kt[:, c * KC : (c + 1) * KC],
                        rhs=qt[:, qh * QB + s * SL : qh * QB + (s + 1) * SL],
                        start=True,
                        stop=True,
                    )
                pt = pt_pool.tile([KC, QB], BF16)
                if _unit_on_scalar(u):
                    nc.scalar.activation(
                        out=pt[:, :],
                        in_=st[:, :],
                        func=mybir.ActivationFunctionType.Exp,
                    )
                else:
                    nc.vector.tensor_scalar(
                        pt[:, :].bitcast(I16),
                        st[:, :],
                        SCH_A,
                        SCH_B,
                        mybir.AluOpType.mult,
                        mybir.AluOpType.add,
                    )
                # PSUM `start` zeroes a whole 2KB bank (zero region), so the
                # 8 j-groups sharing one OT bank form a single accumulation
                # group: start only on the bank's very first matmul, stop on
                # its last. Same for the shared den bank.
                for j in range(NJ):
                    pts = pt[:, j * KC : (j + 1) * KC]
                    nc.tensor.matmul(
                        out=ot[qh][:, j * D : (j + 1) * D],
                        lhsT=pts,
                        rhs=vr[:, c, :],
                        start=(c == 0 and j == 0),
                        stop=(c == NKC - 1 and j == NJ - 1),
                        skip_group_check=True,
                    )
                    nc.tensor.matmul(
                        out=den_ps[:, qh * NJ + j : qh * NJ + j + 1],
                        lhsT=pts,
                        rhs=ones[:, :],
                        start=(u == 0 and j == 0),
                        stop=(u == NQB * NKC - 1 and j == NJ - 1),
                        skip_group_check=True,
                    )
            # Ship the finished q-half (normalization happens on host).
            ob = ob_pool.tile([KC, NJ * D], F32)
            nc.scalar.activation(
                out=ob[:, :], in_=ot[qh][:, :],
                func=mybir.ActivationFunctionType.Copy,
            )
            nc.sync.dma_start(
                out=o_d[:, qh * NJ * D : (qh + 1) * NJ * D], in_=ob[:, :]
            )
        den_sb = singles.tile([KC, NQB * NJ], F32)
        nc.vector.tensor_copy(den_sb[:, :], den_ps[:, :])
        nc.sync.dma_start(out=den_d[:, :], in_=den_sb[:, :])

    nc.finalize()
    return nc


_PROGRAM_CACHE = {}


def _get_program():
    if "nc" not in _PROGRAM_CACHE:
        _PROGRAM_CACHE["nc"] = _build_program()
    return _PROGRAM_CACHE["nc"]


def _make_in_maps(Q, K, V):
    Q = np.asarray(Q, dtype=np.float32)
    K = np.asarray(K, dtype=np.float32)
    V = np.asarray(V, dtype=np.float32)
    in_maps = []
    for core in range(N_CORES):
        b, half = core // 2, core % 2
        q_shard = Q[b, half * LQ_SHARD : (half + 1) * LQ_SHARD, :]  # [2048, 64]
        qt = np.ascontiguousarray(q_shard.T).astype(np.float16)  # [64, 2048]
        kt = np.ascontiguousarray(K[b].T).astype(np.float16)  # [64, 4096]
        # VR[p, c, d] = V[c*128 + p, d]
        vr = np.ascontiguousarray(
            V[b].reshape(NKC, KC, D).swapaxes(0, 1)
        ).astype(BF16NP)
        in_maps.append({"QT": qt, "KT": kt, "VR": vr})
    return in_maps


def _run(Q, K, V, trace=False, **spmd_kwargs):
    nc = _get_program()
    in_maps = _make_in_maps(Q, K, V)
    res = run_bass_kernel_spmd(
        nc, in_maps, list(range(N_CORES)), trace=trace, **spmd_kwargs
    )
    out = np.empty((B, LQ, D), dtype=np.float32)
    for core in range(N_CORES):
        b, half = core // 2, core % 2
        o = res.results[core]["O"].reshape(KC, NQB * NJ, D)  # [p, j, d]
        den = res.results[core]["DEN"]  # [p, j]
        shard = (o / den[:, :, None]).swapaxes(0, 1).reshape(LQ_SHARD, D)
        out[b, half * LQ_SHARD : (half + 1) * LQ_SHARD, :] = shard
    return out, res


def kernel(Q, K, V):
    out, _ = _run(Q, K, V, trace=False)
    return out
